# revision 7
# baseline (speedup 1.0000x reference)
"""DepthToSpace (cell=4, 4 split groups) Trainium2 Bass kernel.

Full input x: [8, 64, 256, 256] f32 -> output [8, 4, 1024, 1024] f32.
out[b, s, 4h+r, 4w+c] = x[b, 16s + 4r + c, h, w]

Sharding: data parallel over batch — core b handles x[b] (16.8 MB in/out).

Per-core plan (pure data movement, memory-bound): partition p = h//2.
All DMAs issue from the Sync engine onto one HWDGE ring: loads first,
stores queue strictly behind them, so the HBM stream is a pure-read
phase followed by a pure-write phase (no read/write turnaround mixing).

Loads are split per channel: 64 dma_starts of 256KB. Each DMA's 128
descriptors (2KB each) cover one contiguous 256KB channel, so each SDMA
engine's consecutive descriptors cluster into contiguous 8KB runs —
sequential HBM read streams. This lifted the measured load phase from
~293 GB/s (4MB channel-major DMAs whose per-engine descriptors jump
256KB apart) to ~410-416 GB/s. Larger ch-per-DMA (2) mis-executes
(wrong results) despite correct BIR APs — keep 1.

Per split group s:
  load   : 16x X[p, ch, h2, w] = x[16s+ch, 2p+h2, w]
  shuffle: Y[p, h2, r, w, c] = X[p, 4r+c, h2, w]  (strided copies),
           split DVE:ACT ~= 5:3 by elements to balance engine rates
  store  : Y -> y[s] rows 8p+4h2+r, cols 4w+c — a fully contiguous 4MB
           region (32KB/partition descriptors). An h2-split (16KB descs)
           measured ~3µs faster drains but corrupted output in 1 of 2
           full-wait runs — not worth the risk.
Stores on the scalar (ACT HWDGE) ring mis-execute AND are slower —
keep everything on the Sync ring. GPSIMD/SWDGE also measured slower.

Measured: 95235 ns, rel err 0.0 (vs 114454 ns for the staged baseline
on the same machine/day; graded baseline was 106665 ns). Early-release
final waits and h2-split stores measured faster still (71-84 µs) but
intermittently corrupted output — rejected.
"""

import sys

sys.path.insert(0, "/opt/trn_rl_repo")

import numpy as np

import concourse.bass as bass
import concourse.mybir as mybir
from concourse.bass_utils import run_bass_kernel_spmd

B, C, H, W = 8, 64, 256, 256
S = 4
CELL = 4  # sqrt(C // S)
CPG = C // S  # channels per group = 16
P = 128  # SBUF partitions
HB = H // P  # h rows per partition = 2
N_CORES = 8

NXB = 3  # X buffers
NYB = 3  # Y buffers

STORE_SPLIT = 1  # one dma_start per group store (proven structure)
# Cap store descriptor runs at 16KB (4096 f32) instead of the natural
# 32KB: measured store phase 34-36µs (~465 GB/s) vs 43.6µs (385 GB/s).
# Applied via balance_dma_aps' last-dim splitter inside the SAME single
# dma_start, so semaphore/instruction structure is unchanged.
STORE_LAST_DIM_BYTES = 16384

# Shuffle work units (h2, r_lo, r_hi) — DVE gets h2=0 all r + h2=1 r0;
# ACT gets h2=1 r1..r3.
DVE_UNITS = [(0, 0, 4), (1, 0, 1)]
ACT_UNITS = [(1, 1, 4)]


def build_program():
    nc = bass.Bass()
    x = nc.declare_dram_parameter("x", [C, H, W], mybir.dt.float32, isOutput=False)
    y = nc.declare_dram_parameter(
        "y", [S, H * CELL, W * CELL], mybir.dt.float32, isOutput=True
    )

    from contextlib import ExitStack

    with ExitStack() as ctx:
        sb = lambda name, shape: ctx.enter_context(
            nc.sbuf_tensor(name, shape, mybir.dt.float32)
        )
        sem = lambda name: ctx.enter_context(nc.semaphore(name))
        Xt = [sb(f"X{i}", [P, CPG, HB, W]) for i in range(NXB)]
        Yt = [sb(f"Y{i}", [P, HB, CELL, W, CELL]) for i in range(NYB)]
        inl = sem("inl")  # one monotonic counter: +16 per channel DMA
        outs = sem("outs")  # +16 per store DMA
        shuf_v = sem("shuf_v")
        shuf_a = sem("shuf_a")
        block = ctx.enter_context(nc.Block())

        LOAD_INCS = 16 * CPG  # sem value added per fully-loaded group
        STORE_INCS = 16 * STORE_SPLIT

        def load_ap(s, ch):
            # one channel: 128 descriptors x 2KB covering contiguous 256KB
            return x[s * CPG + ch].rearrange("(p h2) w -> p h2 w", h2=HB)

        def store_ap(s):
            # y[s] as [p, h2, r, w, c]: row = 8p+4h2+r, col = 4w+c.
            return y[s].rearrange(
                "(p h2 r) (w c) -> p h2 r w c", h2=HB, r=CELL, c=CELL
            )

        def copy_aps(Xb, Yb, h2, r_lo, r_hi):
            xr = Xb[:].rearrange("p (r c) h2 w -> p r c h2 w", r=CELL)
            src = xr[:, r_lo:r_hi, :, h2, :]
            dst = Yb[:, h2, r_lo:r_hi].transpose([0, 1, 3, 2])
            return src, dst

        n_dve = len(DVE_UNITS)
        n_act = len(ACT_UNITS)

        @block.sync
        def _(sync):
            for s in range(S):
                if s >= NXB:
                    # X[s%NXB] free once shuffle(s-NXB) fully done
                    sync.wait_ge(shuf_v, n_dve * (s - NXB + 1))
                    sync.wait_ge(shuf_a, n_act * (s - NXB + 1))
                for ch in range(CPG):
                    sync.dma_start(
                        out=Xt[s % NXB][:, ch], in_=load_ap(s, ch)
                    ).then_inc(inl, 16)
            # Stores queue behind all loads on this ring: pure-read phase
            # then pure-write phase at full solo DMA bandwidth each.
            for s in range(S):
                sync.wait_ge(shuf_v, n_dve * (s + 1))
                sync.wait_ge(shuf_a, n_act * (s + 1))
                if STORE_SPLIT == 1:
                    sync.dma_start(
                        out=store_ap(s), in_=Yt[s % NYB][:],
                        max_dma_last_dim=STORE_LAST_DIM_BYTES,
                    ).then_inc(outs, 16)
                else:
                    sap = store_ap(s)
                    for h2 in range(HB):
                        sync.dma_start(
                            out=sap[:, h2], in_=Yt[s % NYB][:, h2]
                        ).then_inc(outs, 16)
            # Full completion wait. Do NOT release early (e.g. outs >= 64):
            # the NEFF end-of-execution barrier does NOT reliably wait for
            # DMA-queue drain — an early release measured faster but
            # intermittently returned stale output (host read y before the
            # last stores landed).
            sync.wait_ge(outs, STORE_INCS * S)

        @block.vector
        def _(vector):
            for s in range(S):
                vector.wait_ge(inl, LOAD_INCS * (s + 1))
                if s >= NYB:
                    vector.wait_ge(outs, STORE_INCS * (s - NYB + 1))
                for h2, r_lo, r_hi in DVE_UNITS:
                    src, dst = copy_aps(Xt[s % NXB], Yt[s % NYB], h2, r_lo, r_hi)
                    vector.tensor_copy(out=dst, in_=src).then_inc(shuf_v, 1)

        @block.scalar
        def _(scalar):
            for s in range(S):
                scalar.wait_ge(inl, LOAD_INCS * (s + 1))
                if s >= NYB:
                    scalar.wait_ge(outs, STORE_INCS * (s - NYB + 1))
                for h2, r_lo, r_hi in ACT_UNITS:
                    src, dst = copy_aps(Xt[s % NXB], Yt[s % NYB], h2, r_lo, r_hi)
                    scalar.copy(out=dst, in_=src).then_inc(shuf_a, 1)

    return nc


def run_sharded(x: np.ndarray, trace: bool = False):
    """Shard x over batch across 8 cores, run, gather. Returns (out, results)."""
    assert x.shape == (B, C, H, W), x.shape
    nc = build_program()
    in_maps = [{"x": np.ascontiguousarray(x[b])} for b in range(N_CORES)]
    res = run_bass_kernel_spmd(nc, in_maps, list(range(N_CORES)), trace=trace)
    out = np.stack([res.results[b]["y"] for b in range(N_CORES)], axis=0)
    return out.astype(x.dtype, copy=False), res


def kernel(**inputs: np.ndarray) -> np.ndarray:
    x = np.asarray(inputs["x"], dtype=np.float32)
    out, _ = run_sharded(x, trace=False)
    return out
